# revision 16
# baseline (speedup 1.0000x reference)
"""Trainium2 Bass kernel for nn_GAT_GCN (gnn_message_passing).

Contract: kernel(**inputs) takes FULL unsharded inputs, returns FULL output
[128, 128, 21] float32.

Strategy: pure data-parallel over batch B=128 across 8 cores (16 b / core).
Everything computed on-device per core; weights/graph replicated.

Dataflow per core:
  1. -d distances via one packed block-diag matmul (fp32, exact):
     negS = 2*x1.x2 - |x2|^2  (row-constant |x1|^2 dropped - rank-free).
  2. top-64 per row: 8 rounds of DVE max8 + match_replace, then 8x max_index
     (exact fp32 top-k, order-free downstream).
  3. gather points2/xyz2 columns with GPSIMD ap_gather (channels stay on
     partitions, no transposes).
  4. conv pipeline in fp16 operands (fp32 PSUM accumulate); per-n1 broadcast
     additive terms folded into the PE via a constant 0/1 expand matrix E;
     biases via ACT bias / scalar_tensor_tensor.
"""

import os
import sys

import numpy as np

sys.path.insert(0, "/opt/trn_rl_repo")

import concourse.bass as bass
import concourse.mybir as mybir
import concourse.tile as tile
from concourse import bacc as bacc_mod
from concourse.bass_types import AP

F32 = mybir.dt.float32
F16 = mybir.dt.float16
I16 = mybir.dt.int16
U16 = mybir.dt.uint16
AF = mybir.ActivationFunctionType
ALU = mybir.AluOpType

B, NC = 128, 8
BL = B // NC            # 16 batch elements per core
N1, N2, D, NS = 21, 1024, 128, 64
P = N1 * NS             # 1344 positions, n1-major: p = n1*64 + s

# position chunks (multiples of 64 so every chunk covers whole n1 groups)
CHUNKS = [(0, 512, 0, 8), (512, 1024, 8, 16), (1024, 1344, 16, 21)]
# selection tiles: (b0, b1) -> rows = (b1-b0)*21, padded to 126
SELT = [(0, 6), (6, 12), (12, 16)]

NEG_BIG = -1.0e30


def _bcast_seg(ap2d, r0, r1, rep=NS):
    """[128, N1] AP -> [128, r1-r0, rep] view broadcast along 0-stride inner dim."""
    return ap2d[:, r0:r1].unsqueeze(2).to_broadcast([ap2d.shape[0], r1 - r0, rep])


def build_program():
    nc = bacc_mod.Bacc()

    def din(name, shape, dt=F32):
        return nc.declare_dram_parameter(name, list(shape), dt, isOutput=False)

    p2 = din("p2", (BL, D, N2))           # fp32 (gather needs 4B granularity)
    p1 = din("p1", (BL, D, N1))
    x1 = din("x1", (BL, 3, N1), F16)
    x2 = din("x2", (BL, 3, N2))
    dlhs = din("dlhs", (3, 24, 126))      # dist lhsT per sel tile (2*x1 ; -1)
    drhs = din("drhs", (3, 24, N2))       # dist rhs per sel tile (x2 ; |x2|^2)
    emat = din("emat", (N1, P), F16)      # E[n1, p] = 1 if p//64 == n1
    aflat = din("aflat", (D, N1 * N1))    # graph_a[c, m, n] flat (n contiguous)
    wnames = [
        "qw0T", "kw0T", "fv1T", "qw1T", "kw1T", "fv2T", "vw0T", "vw1T",
        "vw2T", "gwT", "fqT", "fkT", "qw2Ta", "qw2Tb", "kw2Ta", "kw2Tb",
    ]
    wt = {n: din("w_" + n, (D, D), F16) for n in wnames}
    wx = {n: din("wx_" + n, (3, D), F16)
          for n in ["qw0xT", "kw0xT", "fv1xT", "nqw0xT", "nkw0xT", "nfv1xT"]}
    biasv = din("biasv", (D, 8))          # cols: qb0 qb1 kb0 kb1 vb0 vb1 vb2 gwb
    bias2 = din("bias2", (D, 4))          # cols: qb2a qb2b kb2a kb2b

    outp = nc.declare_dram_parameter("outp", [BL, D, N1], F32, isOutput=True)
    iscr = nc.dram_tensor("iscr", [BL, P], I16)  # idx bounce buffer

    with tile.TileContext(nc) as tc:
        import contextlib

        ctx = contextlib.ExitStack()
        with ctx:
            consts = ctx.enter_context(tc.tile_pool(name="consts", bufs=1))
            selp = ctx.enter_context(tc.tile_pool(name="selp", bufs=2))
            v8p = ctx.enter_context(tc.tile_pool(name="v8p", bufs=2))
            idxp = ctx.enter_context(tc.tile_pool(name="idxp", bufs=2))
            perb = ctx.enter_context(tc.tile_pool(name="perb", bufs=2))
            chk = ctx.enter_context(tc.tile_pool(name="chk", bufs=2))
            pp_mm = ctx.enter_context(tc.tile_pool(name="pp_mm", bufs=3, space="PSUM"))
            pp_q3 = ctx.enter_context(tc.tile_pool(name="pp_q3", bufs=2, space="PSUM"))
            pp_p1e = ctx.enter_context(tc.tile_pool(name="pp_p1e", bufs=1, space="PSUM"))
            pp_sm = ctx.enter_context(tc.tile_pool(name="pp_sm", bufs=2, space="PSUM"))

            def cload(dram):
                t = consts.tile(list(dram.shape), dram.dtype, tag="c_" + dram.name)
                nc.sync.dma_start(out=t, in_=dram[:])
                return t

            wt_sb = {n: cload(wt[n]) for n in wnames}
            wx_sb = {n: cload(wx[n]) for n in wx}
            emat_sb = cload(emat)
            aflat_sb = cload(aflat)
            biasv_sb = cload(biasv)
            bias2_sb = cload(bias2)
            alpha_sb = consts.tile([D, 1], F32, tag="alpha")
            nc.vector.memset(alpha_sb, 0.1)
            # static 16-row fp32 source for the xyz gather (rows 3.. pad)
            xsrc = consts.tile([16, N2], F32, tag="xsrc")
            nc.vector.memset(xsrc, 0.0)

            BIAS = {n: biasv_sb[:, i:i + 1]
                    for i, n in enumerate(
                        ["qb0", "qb1", "kb0", "kb1", "vb0", "vb1", "vb2", "gwb"])}
            B2 = {n: bias2_sb[:, i:i + 1]
                  for i, n in enumerate(["qb2a", "qb2b", "kb2a", "kb2b"])}

            # lrelu(x+bias). KERN_LRELU=stt composes it (CoreSim has no Lrelu;
            # also the fallback if HW Lrelu misbehaves).
            lrelu_stt = os.environ.get("KERN_LRELU", "act") == "stt"

            def act_lrelu(out, in_, bias=0.0):
                if not lrelu_stt:
                    # HW Lrelu ignores alpha (acts like relu); Prelu with an
                    # AP alpha is exact (verified on HW).
                    nc.scalar.activation(out, in_, AF.Prelu, bias=bias,
                                         alpha=alpha_sb[:out.shape[0], :])
                else:
                    nc.scalar.activation(out, in_, AF.Identity, bias=bias)
                    nc.vector.scalar_tensor_tensor(
                        out, out, 0.1, out, op0=ALU.mult, op1=ALU.max)

            # ================= Phase 1: distances + top-64 selection =========
            for t in range(3):
                dl = selp.tile([24, 126], F32, tag="dl")
                dr = selp.tile([24, N2], F32, tag="dr")
                nc.sync.dma_start(out=dl, in_=dlhs[t])
                nc.sync.dma_start(out=dr, in_=drhs[t])
                negs = selp.tile([126, N2], F32, tag="negs")
                for h in range(2):
                    ps = pp_mm.tile([126, 512], F32, tag="mm")
                    nc.tensor.matmul(ps, lhsT=dl, rhs=dr[:, h * 512:(h + 1) * 512],
                                     start=True, stop=True)
                    nc.scalar.activation(negs[:, h * 512:(h + 1) * 512], ps, AF.Copy)
                work = selp.tile([126, N2], F32, tag="work")
                v8s = []
                cur = negs
                for rnd in range(8):
                    v8 = v8p.tile([126, 8], F32, tag=f"v8_{rnd}")
                    nc.vector.max(out=v8, in_=cur)
                    nc.vector.match_replace(out=work, in_to_replace=v8,
                                            in_values=cur, imm_value=NEG_BIG)
                    v8s.append(v8)
                    cur = work
                idxt = idxp.tile([126, 64], U16, tag="idxt")
                for rnd in range(8):
                    nc.vector.max_index(idxt[:, rnd * 8:(rnd + 1) * 8], v8s[rnd], negs)
                b0, b1 = SELT[t]
                for b in range(b0, b1):
                    rr = (b - b0) * N1
                    nc.sync.dma_start(
                        out=iscr[b].rearrange("(a c) -> a c", c=64),
                        in_=idxt[rr:rr + N1, :].bitcast(I16))

            # ================= Phase 2: per-batch-element pipeline ===========
            for b in range(int(os.environ.get("KERN_NB", BL))):
                p2b = perb.tile([D, N2], F32, tag="p2b")
                nc.sync.dma_start(out=p2b, in_=p2[b])
                p1b = perb.tile([D, N1], F32, tag="p1b")
                nc.sync.dma_start(out=p1b, in_=p1[b])
                p1b16 = perb.tile([D, N1], F16, tag="p1b16")
                nc.scalar.activation(p1b16, p1b, AF.Copy)
                x1b = perb.tile([3, N1], F16, tag="x1b")
                nc.sync.dma_start(out=x1b, in_=x1[b])
                nc.sync.dma_start(out=xsrc[0:3, :], in_=x2[b])
                idxb = perb.tile([D, P // 16], I16, tag="idxb")
                ib = iscr[b]
                for g in range(8):
                    nc.sync.dma_start(
                        out=idxb[g * 16:(g + 1) * 16, :],
                        in_=AP(tensor=ib.tensor, offset=ib.offset,
                               ap=[[1, 16], [16, P // 16]]))

                # -- gathers (GPSIMD), then cast to fp16 --
                G = perb.tile([D, P], F32, tag="G")
                nc.gpsimd.ap_gather(G, p2b, idxb, channels=D, num_elems=N2,
                                    d=1, num_idxs=P)
                XG = perb.tile([16, P], F32, tag="XG")
                nc.gpsimd.ap_gather(XG, xsrc, idxb[0:16, :], channels=16,
                                    num_elems=N2, d=1, num_idxs=P)
                G16 = perb.tile([D, P], F16, tag="G16")
                nc.scalar.activation(G16, G, AF.Copy)
                XG16 = perb.tile([3, P], F16, tag="XG16")
                nc.scalar.activation(XG16, XG[0:3, :], AF.Copy)

                # -- graph branch: pg[c,m] = sum_n p1[c,n] * A[c,n,m] --
                pgm = perb.tile([D, N1 * N1], F32, tag="pgm")
                nc.vector.tensor_mul(
                    pgm.rearrange("c (m n) -> c m n", n=N1),
                    aflat_sb.rearrange("c (m n) -> c m n", n=N1),
                    p1b.unsqueeze(1).to_broadcast([D, N1, N1]))
                pg = perb.tile([D, N1], F16, tag="pg")
                with nc.allow_low_precision(reason="fp16 matmul operand; DVE accumulates fp32 internally"):
                    nc.vector.tensor_reduce(
                        pg, pgm.rearrange("c (m n) -> c m n", n=N1),
                        axis=mybir.AxisListType.X, op=ALU.add)
                psg = pp_sm.tile([D, N1], F32, tag="sm")
                nc.tensor.matmul(psg, lhsT=wt_sb["gwT"], rhs=pg,
                                 start=True, stop=True)
                p1g = perb.tile([D, N1], F16, tag="p1g")
                act_lrelu(p1g, psg, bias=BIAS["gwb"])

                # -- E-fold lhsT builders: [21, 128] = (term)^T in fp16 --
                def efold(parts, tag):
                    ps = pp_sm.tile([N1, D], F32, tag="sm")
                    n = len(parts)
                    for i, (lh, rh) in enumerate(parts):
                        nc.tensor.matmul(ps, lhsT=lh, rhs=rh,
                                         start=(i == 0), stop=(i == n - 1))
                    sb = perb.tile([N1, D], F16, tag=tag)
                    nc.scalar.activation(sb, ps, AF.Copy)
                    return sb

                efq = efold([(p1g, wt_sb["fqT"]), (x1b, wx_sb["nqw0xT"])], "efq")
                efk = efold([(p1g, wt_sb["fkT"]), (x1b, wx_sb["nkw0xT"])], "efk")
                efv = efold([(x1b, wx_sb["nfv1xT"])], "efv")
                efv0 = efold([(p1b16, wt_sb["vw0T"])], "efv0")  # (vw0 @ p1b)^T

                def conv0(ps, w, wxk, ef, c0, c1):
                    nc.tensor.matmul(ps, lhsT=wt_sb[w], rhs=G16[:, c0:c1],
                                     start=True, stop=False)
                    nc.tensor.matmul(ps, lhsT=wx_sb[wxk], rhs=XG16[:, c0:c1],
                                     start=False, stop=False)
                    nc.tensor.matmul(ps, lhsT=ef, rhs=emat_sb[:, c0:c1],
                                     start=False, stop=True)

                # -- K path (all chunks) -> k2sb, kmax, k3 --
                k2sb = perb.tile([D, P], F16, tag="k2sb")
                for (c0, c1, r0, r1) in CHUNKS:
                    n = c1 - c0
                    psk1 = pp_mm.tile([D, 512], F32, tag="mm")
                    conv0(psk1[:, :n], "kw0T", "kw0xT", efk, c0, c1)
                    k1 = chk.tile([D, 512], F16, tag="k1")
                    act_lrelu(k1[:, :n], psk1[:, :n], bias=BIAS["kb0"])
                    psk2 = pp_mm.tile([D, 512], F32, tag="mm")
                    nc.tensor.matmul(psk2[:, :n], lhsT=wt_sb["kw1T"],
                                     rhs=k1[:, :n], start=True, stop=True)
                    act_lrelu(k2sb[:, c0:c1], psk2[:, :n], bias=BIAS["kb1"])
                kmax = perb.tile([D, N1], F16, tag="kmax")
                nc.vector.tensor_reduce(
                    kmax, k2sb.rearrange("c (a s) -> c a s", s=NS),
                    axis=mybir.AxisListType.X, op=ALU.max)
                k3 = perb.tile([D, 2, N1], F16, tag="k3")
                for h, (w, bb) in enumerate([("kw2Ta", "kb2a"), ("kw2Tb", "kb2b")]):
                    psk3 = pp_sm.tile([D, N1], F32, tag="sm")
                    nc.tensor.matmul(psk3, lhsT=wt_sb[w], rhs=kmax,
                                     start=True, stop=True)
                    nc.scalar.activation(k3[:, h, :], psk3, AF.Identity, bias=B2[bb])

                # -- Q + V path per chunk --
                vres = perb.tile([D, N1], F32, tag="vres")
                vmax = perb.tile([D, N1], F16, tag="vmax")
                for (c0, c1, r0, r1) in CHUNKS:
                    n = c1 - c0
                    psq1 = pp_mm.tile([D, 512], F32, tag="mm")
                    conv0(psq1[:, :n], "qw0T", "qw0xT", efq, c0, c1)
                    q1 = chk.tile([D, 512], F16, tag="q1")
                    act_lrelu(q1[:, :n], psq1[:, :n], bias=BIAS["qb0"])
                    psq2 = pp_mm.tile([D, 512], F32, tag="mm")
                    nc.tensor.matmul(psq2[:, :n], lhsT=wt_sb["qw1T"],
                                     rhs=q1[:, :n], start=True, stop=True)
                    q2 = chk.tile([D, 512], F16, tag="q2")
                    act_lrelu(q2[:, :n], psq2[:, :n], bias=BIAS["qb1"])
                    # v = fv1 conv (no bias) - stays in PSUM
                    psv = pp_mm.tile([D, 512], F32, tag="mm")
                    conv0(psv[:, :n], "fv1T", "fv1xT", efv, c0, c1)
                    # q3 halves in PSUM; a = sigmoid((q3+qb2) * k3bcast)
                    g12 = []
                    for h, bb in enumerate(["qb2a", "qb2b"]):
                        psq3 = pp_q3.tile([D, 512], F32, tag="q3")
                        nc.tensor.matmul(psq3[:, :n],
                                         lhsT=wt_sb["qw2Ta" if h == 0 else "qw2Tb"],
                                         rhs=q2[:, :n], start=True, stop=True)
                        ga = chk.tile([D, 512], F16, tag=f"ga_{h}")
                        nc.vector.scalar_tensor_tensor(
                            ga[:, :n].rearrange("c (a s) -> c a s", s=NS),
                            psq3[:, :n].rearrange("c (a s) -> c a s", s=NS),
                            B2[bb], _bcast_seg(k3[:, h, :], r0, r1),
                            op0=ALU.add, op1=ALU.mult)
                        g = chk.tile([D, 512], F16, tag=f"g_{h}")
                        nc.scalar.activation(g[:, :n], ga[:, :n], AF.Sigmoid)
                        g12.append(g)
                    g1, g2 = g12
                    # p1e = fv2 @ (g2 * p1g_bcast)
                    tpe = chk.tile([D, 512], F16, tag="tpe")
                    nc.vector.tensor_mul(
                        tpe[:, :n].rearrange("c (a s) -> c a s", s=NS),
                        g2[:, :n].rearrange("c (a s) -> c a s", s=NS),
                        _bcast_seg(p1g, r0, r1))
                    psp1e = pp_p1e.tile([D, 512], F32, tag="p1e")
                    nc.tensor.matmul(psp1e[:, :n], lhsT=wt_sb["fv2T"],
                                     rhs=tpe[:, :n], start=True, stop=True)
                    # v1 = lrelu(v * g1 + p1e)   (+p1b deferred)
                    tv1 = chk.tile([D, 512], F16, tag="tv1")
                    nc.vector.scalar_tensor_tensor(
                        tv1[:, :n], psv[:, :n], 0.0, g1[:, :n],
                        op0=ALU.add, op1=ALU.mult)
                    tv1b = chk.tile([D, 512], F16, tag="tv1b")
                    nc.vector.tensor_add(tv1b[:, :n], tv1[:, :n], psp1e[:, :n])
                    v1 = chk.tile([D, 512], F16, tag="v1")
                    act_lrelu(v1[:, :n], tv1b[:, :n])
                    nc.vector.tensor_reduce(
                        vres[:, r0:r1], v1[:, :n].rearrange("c (a s) -> c a s", s=NS),
                        axis=mybir.AxisListType.X, op=ALU.add)
                    # v2 = lrelu((vw0@(v1+p1b) + vb0) * g1 + p1e)
                    psv2 = pp_mm.tile([D, 512], F32, tag="mm")
                    nc.tensor.matmul(psv2[:, :n], lhsT=wt_sb["vw0T"],
                                     rhs=v1[:, :n], start=True, stop=False)
                    nc.tensor.matmul(psv2[:, :n], lhsT=efv0,
                                     rhs=emat_sb[:, c0:c1], start=False, stop=True)
                    tv2 = chk.tile([D, 512], F16, tag="tv2")
                    nc.vector.scalar_tensor_tensor(
                        tv2[:, :n], psv2[:, :n], BIAS["vb0"], g1[:, :n],
                        op0=ALU.add, op1=ALU.mult)
                    tv2b = chk.tile([D, 512], F16, tag="tv2b")
                    nc.vector.tensor_add(tv2b[:, :n], tv2[:, :n], psp1e[:, :n])
                    v2 = chk.tile([D, 512], F16, tag="v2")
                    act_lrelu(v2[:, :n], tv2b[:, :n])
                    # v3 = lrelu(vw1@v2 + vb1); vmax partial
                    psv3 = pp_mm.tile([D, 512], F32, tag="mm")
                    nc.tensor.matmul(psv3[:, :n], lhsT=wt_sb["vw1T"],
                                     rhs=v2[:, :n], start=True, stop=True)
                    v3 = chk.tile([D, 512], F16, tag="v3")
                    act_lrelu(v3[:, :n], psv3[:, :n], bias=BIAS["vb1"])
                    nc.vector.tensor_reduce(
                        vmax[:, r0:r1], v3[:, :n].rearrange("c (a s) -> c a s", s=NS),
                        axis=mybir.AxisListType.X, op=ALU.max)

                # final: out = lrelu(vw2@vmax + vb2) + vres/64 + p1b
                pso = pp_sm.tile([D, N1], F32, tag="sm")
                nc.tensor.matmul(pso, lhsT=wt_sb["vw2T"], rhs=vmax,
                                 start=True, stop=True)
                vout = perb.tile([D, N1], F32, tag="vout")
                act_lrelu(vout, pso, bias=BIAS["vb2"])
                vr = perb.tile([D, N1], F32, tag="vr")
                nc.vector.scalar_tensor_tensor(
                    vr, vres, 1.0 / NS, vout, op0=ALU.mult, op1=ALU.add)
                osb = perb.tile([D, N1], F32, tag="osb")
                nc.vector.tensor_add(osb, vr, p1b)
                nc.sync.dma_start(out=outp[b], in_=osb)

    if not nc.is_finalized():
        nc.finalize()
    return nc


# ----------------------------------------------------------------------------
# host-side prep + launch
# ----------------------------------------------------------------------------

_NC_CACHE = {}


def _prep_core_inputs(ci, I):
    """Build the per-core input dict for core ci from full inputs I."""
    s = slice(ci * BL, (ci + 1) * BL)
    x1 = I["xyz1"][s].astype(np.float32)      # [BL, 3, 21]
    x2 = I["xyz2"][s].astype(np.float32)      # [BL, 3, 1024]
    dlhs = np.zeros((3, 24, 126), np.float32)
    drhs = np.zeros((3, 24, N2), np.float32)
    for t, (b0, b1) in enumerate(SELT):
        for i, b in enumerate(range(b0, b1)):
            dlhs[t, 4 * i:4 * i + 3, 21 * i:21 * (i + 1)] = 2.0 * x1[b]
            dlhs[t, 4 * i + 3, 21 * i:21 * (i + 1)] = -1.0
            drhs[t, 4 * i:4 * i + 3, :] = x2[b]
            drhs[t, 4 * i + 3, :] = (x2[b] ** 2).sum(axis=0)
    emat = np.zeros((N1, P), np.float16)
    for n in range(N1):
        emat[n, n * NS:(n + 1) * NS] = 1.0
    aflat = np.ascontiguousarray(
        I["graph_a"].transpose(0, 2, 1)).reshape(D, N1 * N1).astype(np.float32)

    T = lambda a: np.ascontiguousarray(np.asarray(a).T).astype(np.float16)
    w = {}
    w["w_qw0T"] = T(I["qw0"][:, :D]); w["wx_qw0xT"] = T(I["qw0"][:, D:])
    w["w_kw0T"] = T(I["kw0"][:, :D]); w["wx_kw0xT"] = T(I["kw0"][:, D:])
    w["w_fv1T"] = T(I["fv1_w"][:, :D]); w["wx_fv1xT"] = T(I["fv1_w"][:, D:])
    for n in ["qw0xT", "kw0xT", "fv1xT"]:
        w["wx_n" + n] = -w["wx_" + n]
    w["w_qw1T"] = T(I["qw1"]); w["w_kw1T"] = T(I["kw1"])
    w["w_fv2T"] = T(I["fv2_w"]); w["w_vw0T"] = T(I["vw0"])
    w["w_vw1T"] = T(I["vw1"]); w["w_vw2T"] = T(I["vw2"])
    w["w_gwT"] = T(I["gw_w"]); w["w_fqT"] = T(I["fq_w"]); w["w_fkT"] = T(I["fk_w"])
    w["w_qw2Ta"] = T(I["qw2"][:D]); w["w_qw2Tb"] = T(I["qw2"][D:])
    w["w_kw2Ta"] = T(I["kw2"][:D]); w["w_kw2Tb"] = T(I["kw2"][D:])
    biasv = np.stack([I["qb0"], I["qb1"], I["kb0"], I["kb1"], I["vb0"],
                      I["vb1"], I["vb2"], I["gw_b"]], axis=1).astype(np.float32)
    bias2 = np.stack([I["qb2"][:D], I["qb2"][D:], I["kb2"][:D], I["kb2"][D:]],
                     axis=1).astype(np.float32)
    return {
        "p2": np.ascontiguousarray(I["points2"][s]).astype(np.float32),
        "p1": np.ascontiguousarray(I["points1"][s]).astype(np.float32),
        "x1": np.ascontiguousarray(x1).astype(np.float16),
        "x2": np.ascontiguousarray(x2),
        "dlhs": dlhs, "drhs": drhs, "emat": emat, "aflat": aflat,
        "biasv": biasv, "bias2": bias2, **w,
    }


def kernel(**inputs) -> np.ndarray:
    from concourse.bass_utils import run_bass_kernel_spmd

    if "nc" not in _NC_CACHE:
        _NC_CACHE["nc"] = build_program()
    nc = _NC_CACHE["nc"]
    in_maps = [_prep_core_inputs(ci, inputs) for ci in range(NC)]
    res = run_bass_kernel_spmd(nc, in_maps, list(range(NC)))
    out = np.concatenate([res.results[ci]["outp"] for ci in range(NC)], axis=0)
    return out.astype(np.float32)
